# revision 16
# baseline (speedup 1.0000x reference)
"""AdaptiveGraphLayer Trainium2 kernel (8-core data parallel).

Per-sample computation (B=4096, N=17, D=1024):
  h   = x @ W^T + b                                  [B, N, D]
  G   = h h^T (per sample), dist = sqrt(sq_i+sq_j-2G_ij)
  sim = exp(-dist/0.1); top-4 mask (stable ties) + self loop
  adj = row-L1-normalize(pose + 0.1*(mask+eye))      [B, N, N]

Sharding: batch split 512 samples/core across 8 NeuronCores, weights
replicated (pure data parallel, no collectives).

Single-core plan (8704 rows = 512 samples x 17), all DMA via SWDGE
(HWDGE DMAs cannot carry sem waits under this compiler), transposes
via the TensorEngine in bf16:
  - r-blocks of 119 rows (7 samples): every stage is 119-aligned so
    Gram packs never straddle a tile boundary.
  - fc matmul in natural layout: lhsT = x^T block (PE-transposed bf16),
    rhs = W^T (PE-transposed once), accumulate f32 in PSUM, bias added
    during the PSUM->SBUF copy; h stored contiguously per r-block.
  - h converted to bf16, PE-transposed to h^T tiles for the per-pack
    Gram (bf16 matmul, f32 accumulate).
  - D = sq_i + sq_j - 2G via two tiny f32 matmuls on the symmetric
    masked Gram (keeps the diagonal exactly 0), sim = exp(-sqrt(D)*10)
    on ScalarE, stable top-4 mask via DVE max8 + match_replace
    (first-occurrence duplicate semantics == jax.lax.top_k stable
    tie-break), blend with pose, L1-normalize (1/(1+gamma) cancels).
"""

import os
import sys

import numpy as np

for _p in ("/opt/trn_rl_repo", "/root/.axon_site/_ro/trn_rl_repo"):
    if os.path.isdir(_p) and _p not in sys.path:
        sys.path.insert(0, _p)

import concourse.bass as bass  # noqa: E402
import concourse.mybir as mybir  # noqa: E402
from concourse import bacc  # noqa: E402
from concourse.tile import TileContext  # noqa: E402

F32 = mybir.dt.float32
BF16 = mybir.dt.bfloat16
ALU = mybir.AluOpType
ACTF = mybir.ActivationFunctionType

B, N, D = 4096, 17, 1024
N_CORES = 8
BS = B // N_CORES            # 512 samples per core
ROWS = BS * N                # 8704 rows per core
KC = D // 128                # 8 contraction / d chunks
PACK = 7                     # samples per pack / r-block
PR = PACK * N                # 119 rows per full r-block
GAMMA = 0.1
TEMP = 0.1


def _rblocks(rows):
    out = []
    r = 0
    while r < rows:
        out.append((r, min(PR, rows - r)))
        r += PR
    return out


def build(rows=ROWS, finalize=True, stages="all"):
    assert rows % N == 0
    nc = bacc.Bacc()

    x_d = nc.declare_dram_parameter("x", [rows, D], F32, isOutput=False)
    pose_d = nc.declare_dram_parameter("pose_adj", [rows, N], F32, isOutput=False)
    w_d = nc.declare_dram_parameter("fc_w", [D, D], F32, isOutput=False)
    b_d = nc.declare_dram_parameter("fc_b", [D], F32, isOutput=False)
    em_d = nc.declare_dram_parameter("eyemask_c", [PR, N], F32, isOutput=False)
    bo_d = nc.declare_dram_parameter("bones_c", [PR, PR], F32, isOutput=False)
    id_d = nc.declare_dram_parameter("ident_c", [128, 128], F32, isOutput=False)
    h_d = nc.declare_dram_parameter("h", [rows, D], F32, isOutput=True)
    adj_d = nc.declare_dram_parameter("adj", [rows, N], F32, isOutput=True)

    rblocks = _rblocks(rows)
    # pose/adj DMA batching: groups of up to 4 equal-size r-blocks
    pgroups = []
    i = 0
    while i < len(rblocks):
        g = [rblocks[i]]
        while (
            len(g) < 4
            and i + len(g) < len(rblocks)
            and rblocks[i + len(g)][1] == g[0][1]
        ):
            g.append(rblocks[i + len(g)])
        pgroups.append(g)
        i += len(g)

    with TileContext(nc) as tc:
        with (
            tc.tile_pool(name="const", bufs=1) as cpool,
            tc.tile_pool(name="xf", bufs=3) as xfpool,
            tc.tile_pool(name="xb", bufs=3) as xbpool,
            tc.tile_pool(name="xt", bufs=2 * KC) as xtpool,
            tc.tile_pool(name="hsb", bufs=3) as hpool,
            tc.tile_pool(name="hb", bufs=3) as hbpool,
            tc.tile_pool(name="ht", bufs=2 * KC) as htpool,
            tc.tile_pool(name="vec", bufs=3) as vpool,
            tc.tile_pool(name="pa", bufs=3) as papool,
            tc.tile_pool(name="pf", bufs=2, space="PSUM") as pfpool,
            tc.tile_pool(name="pt", bufs=2, space="PSUM") as ptpool,
            tc.tile_pool(name="pg", bufs=2, space="PSUM") as pgpool,
            tc.tile_pool(name="pd", bufs=2, space="PSUM") as pdpool,
        ):
            # ---------- one-time constants ----------
            identf = cpool.tile([128, 128], F32, tag="identf")
            nc.gpsimd.dma_start(out=identf[:, :], in_=id_d[:, :])
            ident = cpool.tile([128, 128], BF16, tag="ident")
            nc.vector.tensor_copy(ident[:, :], identf[:, :])

            # W^T in SBUF bf16: load W, convert, PE-transpose 128x128 blocks
            wt = []
            for k in range(KC):
                wtk = cpool.tile([128, D], BF16, tag=f"wt{k}")
                wt.append(wtk)
            for dc in range(KC):
                wf = xfpool.tile([128, D], F32, tag="xf")
                nc.gpsimd.dma_start(
                    out=wf[:, :], in_=w_d[128 * dc : 128 * (dc + 1), :]
                )
                wb = xbpool.tile([128, D], BF16, tag="xb")
                nc.vector.tensor_copy(wb[:, :], wf[:, :])
                for kc in range(KC):
                    pt = ptpool.tile([128, 128], BF16, tag="pt")
                    nc.tensor.transpose(
                        pt[:, 0:128], wb[:, 128 * kc : 128 * (kc + 1)], ident[:, :]
                    )
                    nc.vector.tensor_copy(
                        wt[kc][:, 128 * dc : 128 * (dc + 1)], pt[:, 0:128]
                    )

            # bias replicated to every partition: b_rep[p, f] = b[f]
            b_rep = cpool.tile([128, D], F32, tag="brep")
            nc.gpsimd.dma_start(
                out=b_rep[:, :],
                in_=b_d[:].rearrange("(o d) -> o d", o=1).to_broadcast([128, D]),
            )

            # mask constants (from DRAM params):
            # eyemask[p, j] = (p % 17 == j); bones[q, p] = (q//17 == p//17)
            eyemask = cpool.tile([PR, N], F32, tag="eyemask")
            nc.gpsimd.dma_start(out=eyemask[:, :], in_=em_d[:, :])
            bones = cpool.tile([PR, PR], F32, tag="bones")
            nc.gpsimd.dma_start(out=bones[:, :], in_=bo_d[:, :])
            eye01 = cpool.tile([PR, N], F32, tag="eye01")
            nc.vector.tensor_scalar_mul(eye01[:, :], eyemask[:, :], GAMMA)
            eyem2 = cpool.tile([PR, N], F32, tag="eyem2")
            nc.vector.tensor_scalar_mul(eyem2[:, :], eyemask[:, :], -2.0)

            # ---------- main pipeline over r-blocks ----------
            for grp in pgroups:
                gr0 = grp[0][0]
                gp = grp[0][1]
                gw = len(grp) * N
                if stages == "all":
                    pose4 = papool.tile([PR, 4 * N], F32, tag="pose4")
                    nc.gpsimd.dma_start(
                        out=pose4[0:gp, 0:gw].rearrange(
                            "p (g n) -> p g n", g=len(grp)
                        ),
                        in_=pose_d[gr0 : gr0 + len(grp) * gp, :].rearrange(
                            "(g p) n -> p g n", g=len(grp)
                        ),
                    )
                    adj4 = papool.tile([PR, 4 * N], F32, tag="adj4")

                for bi, (r0, P) in enumerate(grp):
                    # x block -> bf16 -> x^T tiles
                    xf = xfpool.tile([PR, D], F32, tag="xf")
                    nc.gpsimd.dma_start(out=xf[0:P, :], in_=x_d[r0 : r0 + P, :])
                    xb = xbpool.tile([PR, D], BF16, tag="xb")
                    nc.vector.tensor_copy(xb[0:P, :], xf[0:P, :])
                    xt = []
                    for kc in range(KC):
                        pt = ptpool.tile([128, 128], BF16, tag="pt")
                        nc.tensor.transpose(
                            pt[:, 0:P],
                            xb[0:P, 128 * kc : 128 * (kc + 1)],
                            ident[0:P, 0:P],
                        )
                        t = xtpool.tile([128, PR], BF16, tag="xt")
                        nc.vector.tensor_copy(t[:, 0:P], pt[:, 0:P])
                        xt.append(t)

                    # fc matmul + bias
                    h_sb = hpool.tile([PR, D], F32, tag="hsb")
                    for half in range(2):
                        pf = pfpool.tile([PR, 512], F32, tag="pf")
                        for kc in range(KC):
                            nc.tensor.matmul(
                                out=pf[0:P, :],
                                lhsT=xt[kc][:, 0:P],
                                rhs=wt[kc][:, 512 * half : 512 * (half + 1)],
                                start=(kc == 0),
                                stop=(kc == KC - 1),
                            )
                        nc.vector.tensor_tensor(
                            out=h_sb[0:P, 512 * half : 512 * (half + 1)],
                            in0=pf[0:P, :],
                            in1=b_rep[0:P, 512 * half : 512 * (half + 1)],
                            op=ALU.add,
                        )
                    nc.gpsimd.dma_start(out=h_d[r0 : r0 + P, :], in_=h_sb[0:P, :])

                    # h -> bf16 -> h^T tiles
                    hb = hbpool.tile([PR, D], BF16, tag="hb")
                    nc.vector.tensor_copy(hb[0:P, :], h_sb[0:P, :])
                    ht = []
                    for dc in range(KC):
                        pt = ptpool.tile([128, 128], BF16, tag="pt")
                        nc.tensor.transpose(
                            pt[:, 0:P],
                            hb[0:P, 128 * dc : 128 * (dc + 1)],
                            ident[0:P, 0:P],
                        )
                        t = htpool.tile([128, PR], BF16, tag="ht")
                        nc.vector.tensor_copy(t[:, 0:P], pt[:, 0:P])
                        ht.append(t)

                    if stages == "fc":
                        continue
                    # pack Gram (within-sample blocks used only)
                    pg = pgpool.tile([PR, PR], F32, tag="pg")
                    for dc in range(KC):
                        nc.tensor.matmul(
                            out=pg[0:P, 0:P],
                            lhsT=ht[dc][:, 0:P],
                            rhs=ht[dc][:, 0:P],
                            start=(dc == 0),
                            stop=(dc == KC - 1),
                        )

                    if stages == "gram":
                        continue
                    # masked Gram is symmetric -> usable as stationary:
                    # Gd = masked @ eyemask extracts the diagonal blocks.
                    masked = vpool.tile([PR, PR], F32, tag="masked")
                    nc.vector.tensor_tensor(
                        out=masked[0:P, 0:P],
                        in0=pg[0:P, 0:P],
                        in1=bones[0:P, 0:P],
                        op=ALU.mult,
                    )
                    pgd = pdpool.tile([PR, N], F32, tag="pd")
                    nc.tensor.matmul(
                        out=pgd[0:P, :],
                        lhsT=masked[0:P, 0:P],
                        rhs=eyemask[0:P, :],
                        start=True,
                        stop=True,
                    )
                    mgd = vpool.tile([PR, N], F32, tag="mgd")
                    sq = vpool.tile([PR, 1], F32, tag="sq")
                    nc.vector.tensor_tensor(
                        out=mgd[0:P, :],
                        in0=pgd[0:P, :],
                        in1=eyemask[0:P, :],
                        op=ALU.mult,
                    )
                    nc.vector.tensor_reduce(
                        out=sq[0:P, :],
                        in_=mgd[0:P, :],
                        axis=mybir.AxisListType.X,
                        op=ALU.add,
                    )
                    # psumD = sqT - 2*Gd ; D = psumD + sq_i (diag exactly 0)
                    pd = pdpool.tile([PR, N], F32, tag="pd")
                    nc.tensor.matmul(
                        out=pd[0:P, :],
                        lhsT=bones[0:P, 0:P],
                        rhs=mgd[0:P, :],
                        start=True,
                        stop=False,
                    )
                    nc.tensor.matmul(
                        out=pd[0:P, :],
                        lhsT=masked[0:P, 0:P],
                        rhs=eyem2[0:P, :],
                        start=False,
                        stop=True,
                    )
                    dt = vpool.tile([PR, N], F32, tag="dt")
                    nc.vector.tensor_tensor(
                        out=dt[0:P, :],
                        in0=pd[0:P, :],
                        in1=sq[0:P, :].to_broadcast([P, N]),
                        op=ALU.add,
                    )
                    dist = vpool.tile([PR, N], F32, tag="dist")
                    nc.scalar.activation(dist[0:P, :], dt[0:P, :], ACTF.Sqrt)
                    sim = vpool.tile([PR, N], F32, tag="sim")
                    nc.scalar.activation(
                        sim[0:P, :], dist[0:P, :], ACTF.Exp, scale=-1.0 / TEMP
                    )

                    if stages == "sim":
                        continue
                    # stable top-4 mask
                    needles = vpool.tile([PR, 8], F32, tag="needles")
                    nc.vector.max(out=needles[0:P, :], in_=sim[0:P, :])
                    nc.vector.memset(needles[0:P, 4:8], -1.0)
                    repl = vpool.tile([PR, N], F32, tag="repl")
                    nc.vector.match_replace(
                        out=repl[0:P, :],
                        in_to_replace=needles[0:P, :],
                        in_values=sim[0:P, :],
                        imm_value=-1.0,
                    )
                    mask = vpool.tile([PR, N], F32, tag="mask")
                    nc.vector.tensor_tensor(
                        out=mask[0:P, :],
                        in0=sim[0:P, :],
                        in1=repl[0:P, :],
                        op=ALU.subtract,
                    )
                    # selected entries (>=1) -> GAMMA; others 0
                    nc.vector.tensor_scalar(
                        out=mask[0:P, :],
                        in0=mask[0:P, :],
                        scalar1=GAMMA,
                        scalar2=GAMMA,
                        op0=ALU.mult,
                        op1=ALU.min,
                    )
                    nc.vector.tensor_tensor(
                        out=mask[0:P, :],
                        in0=mask[0:P, :],
                        in1=eye01[0:P, :],
                        op=ALU.add,
                    )
                    u = vpool.tile([PR, N], F32, tag="u")
                    nc.vector.tensor_tensor(
                        out=u[0:P, :],
                        in0=mask[0:P, :],
                        in1=pose4[0:P, N * bi : N * (bi + 1)],
                        op=ALU.add,
                    )
                    rs = vpool.tile([PR, 1], F32, tag="rs")
                    nc.vector.tensor_reduce(
                        out=rs[0:P, :],
                        in_=u[0:P, :],
                        axis=mybir.AxisListType.X,
                        op=ALU.add,
                    )
                    nc.vector.tensor_scalar_max(rs[0:P, :], rs[0:P, :], 1e-12)
                    rsi = vpool.tile([PR, 1], F32, tag="rsi")
                    nc.vector.reciprocal(out=rsi[0:P, :], in_=rs[0:P, :])
                    nc.vector.tensor_tensor(
                        out=adj4[0:P, N * bi : N * (bi + 1)],
                        in0=u[0:P, :],
                        in1=rsi[0:P, :].to_broadcast([P, N]),
                        op=ALU.mult,
                    )

                if stages == "all":
                    nc.gpsimd.dma_start(
                        out=adj_d[gr0 : gr0 + len(grp) * gp, :].rearrange(
                            "(g p) n -> p g n", g=len(grp)
                        ),
                        in_=adj4[0:gp, 0:gw].rearrange(
                            "p (g n) -> p g n", g=len(grp)
                        ),
                    )

    if finalize:
        nc.finalize()
    return nc


def mask_consts():
    p = np.arange(PR)
    em = (p[:, None] % N == np.arange(N)[None, :]).astype(np.float32)
    bo = (p[:, None] // N == p[None, :] // N).astype(np.float32)
    ident = np.eye(128, dtype=np.float32)
    return (
        np.ascontiguousarray(em),
        np.ascontiguousarray(bo),
        np.ascontiguousarray(ident),
    )


def kernel(**inputs):
    x = np.ascontiguousarray(np.asarray(inputs["x"], dtype=np.float32))
    pose = np.ascontiguousarray(np.asarray(inputs["pose_adj"], dtype=np.float32))
    w = np.ascontiguousarray(np.asarray(inputs["fc_w"], dtype=np.float32))
    b = np.ascontiguousarray(np.asarray(inputs["fc_b"], dtype=np.float32))

    from concourse.bass_utils import run_bass_kernel_spmd

    nc = build()
    em, bo, ident = mask_consts()
    in_maps = [
        {
            "x": x[c * BS : (c + 1) * BS].reshape(ROWS, D),
            "pose_adj": pose[c * BS : (c + 1) * BS].reshape(ROWS, N),
            "fc_w": w,
            "fc_b": b,
            "eyemask_c": em,
            "bones_c": bo,
            "ident_c": ident,
        }
        for c in range(N_CORES)
    ]
    res = run_bass_kernel_spmd(nc, in_maps, core_ids=list(range(N_CORES)))
    h = np.concatenate(
        [res.results[c]["h"].reshape(BS, N, D) for c in range(N_CORES)], axis=0
    )
    adj = np.concatenate(
        [res.results[c]["adj"].reshape(BS, N, N) for c in range(N_CORES)], axis=0
    )
    return h, adj
